# revision 14
# baseline (speedup 1.0000x reference)
"""Dual-phase bipartite GATv2 on 8 TRN2 NeuronCores.

Per-core SPMD strategy:
  - Dense phase: project node features once (f32r matmuls, bf16 tables).
      users: each core projects its 12.5k rows -> um-src table (AllGathered
      to a global bf16 table) + mu-dst table (stays local).
      movies: every core redundantly projects all 20k rows (cheap).
  - Edge phases (one per GAT direction): edges sorted by destination and
    packed into 128-edge tiles holding whole destination segments; projected
    rows fetched with batched int16 dma_gather; leaky/att/exp computed in
    batch-wide DVE/ACT ops; segment sums via one-hot (sel) matmuls
    accumulated in PSUM over tile groups; per-node sums + softmax
    denominators scattered with dma_scatter_add into zero-initialized stage
    tables. The um direction splits edges by 25k src chunk so gather indices
    fit int16 (4 partial stages summed in post).
  - Post: normalize (1/den, head mean, +bias) -> u_sat/u_dis/m_sat/m_dis;
    u_final/m_final; y-MLP halves a = u_final@Wp1[:64], b = m_final@Wp1[64:]
    + bp1 (b AllGathered).
  - Y phase: y = relu(a[src]+b[dst]) @ Wp2 (+bp2 on host) over user-sorted
    edges (a local, b global, int16 gathers).
Host does index preprocessing only (sorting / packing / int16 index streams).
"""
import os
import numpy as np
import ml_dtypes
import concourse.bass as bass
import concourse.bacc as bacc
import concourse.mybir as mybir
import concourse.tile as tile
from concourse.bass_utils import run_bass_kernel_spmd
import sys
kernel = sys.modules[__name__]
last_exec_time_ns = None
last_profile = None

NU, NM, E = 100000, 20000, 500000
D, H, C, HC = 128, 2, 64, 128
NC_ = 8
UR = NU // NC_          # users per core (12500)
MR = NM // NC_          # movies per core (2500)
NCH = 4
UCH = NU // NCH         # users per chunk (25000)
P = 128
TRASH = 128

UM_GROUP, UM_BATCH = 4, 8
MU_GROUP, MU_BATCH = 4, 8
Y_BATCH = 8

f32 = mybir.dt.float32
f32r = mybir.dt.float32r
bf16 = mybir.dt.bfloat16
i16 = mybir.dt.int16
AF = mybir.ActivationFunctionType
OP = mybir.AluOpType
BF = ml_dtypes.bfloat16


# ---------------------------------------------------------------- host packing

def _wrap16(idx):
    """[n] -> [128, n/16] int16 wrapped layout (idx i at [i%16, i//16], tiled x8)."""
    n = idx.shape[0]
    assert n % 16 == 0
    w = idx.reshape(n // 16, 16).T.astype(np.int16)
    return np.tile(w, (8, 1))


def _segments(e_dst_local):
    n = e_dst_local.shape[0]
    if n == 0:
        return np.zeros(0, np.int64), np.zeros(0, np.int64)
    bnd = np.flatnonzero(np.diff(e_dst_local)) + 1
    return (np.concatenate([[0], bnd]), np.concatenate([bnd, [n]]))


def _tile_plan(seg_start, seg_end, group):
    """Greedy pack whole segments into 128-edge tiles; <=127 ranks per
    `group`-tile group. Returns list of tiles (each a list of seg ids)."""
    tiles, cur, cur_edges, grp_rank = [], [], 0, 0
    for s in range(seg_start.shape[0]):
        ln = seg_end[s] - seg_start[s]
        assert ln <= 128, f"segment of {ln} edges exceeds a tile"
        if cur_edges + ln > 128 or grp_rank + 1 > 127:
            tiles.append(cur)
            cur, cur_edges = [], 0
            if len(tiles) % group == 0:
                grp_rank = 0
            elif grp_rank + 1 > 127:
                while len(tiles) % group != 0:
                    tiles.append([])
                grp_rank = 0
        cur.append(s)
        cur_edges += ln
        grp_rank += 1
    if cur:
        tiles.append(cur)
    return tiles


def _pack_direction(e_src, e_dst_local, n_tiles_pad, group, n_dst_local):
    """Build per-tile gather/sel/scatter arrays (see module docstring)."""
    seg_start, seg_end = _segments(e_dst_local)
    tiles = _tile_plan(seg_start, seg_end, group)
    T = len(tiles)
    assert T <= n_tiles_pad, (T, n_tiles_pad)

    gsrc = np.zeros((n_tiles_pad, 128), np.int64)
    gdst = np.zeros((n_tiles_pad, 128), np.int64)
    rank = np.full((n_tiles_pad, 128), 127, np.int64)
    n_groups = n_tiles_pad // group
    scat = n_dst_local + np.tile(np.arange(128), (n_groups, 1))
    g_cnt = np.zeros(n_groups, np.int64)
    for t, segs in enumerate(tiles):
        g = t // group
        pos = 0
        for s in segs:
            a, b = seg_start[s], seg_end[s]
            ln = b - a
            q = g_cnt[g]
            g_cnt[g] += 1
            scat[g, q] = e_dst_local[a]
            gsrc[t, pos:pos + ln] = e_src[a:b]
            gdst[t, pos:pos + ln] = e_dst_local[a:b]
            rank[t, pos:pos + ln] = q
            pos += ln
    sel = np.zeros((n_tiles_pad, 128, 128), BF)
    sel[np.arange(n_tiles_pad)[:, None], np.arange(128)[None, :], rank] = BF(1.0)
    return gsrc, gdst, sel, scat


def _count_tiles(e_dst_local, group):
    s, e = _segments(e_dst_local)
    return len(_tile_plan(s, e, group))


def _prep(src, dst):
    src = np.asarray(src).astype(np.int64)
    dst = np.asarray(dst).astype(np.int64)

    # ---------------- um: dst=movies, per (core, chunk) ---------------------
    mcore = dst // MR
    uchunk = src // UCH
    key = (mcore * NCH + uchunk) * NM + dst
    o_um = np.argsort(key, kind="stable")
    s_s, d_s, mc_s, uc_s = src[o_um], dst[o_um], mcore[o_um], uchunk[o_um]
    um_e = {}
    maxT_um = 0
    for k in range(NC_):
        for c in range(NCH):
            m = (mc_s == k) & (uc_s == c)
            um_e[(k, c)] = (s_s[m] - c * UCH, d_s[m] - k * MR)
            maxT_um = max(maxT_um, _count_tiles(um_e[(k, c)][1], UM_GROUP))
    T_UM = -(-maxT_um // UM_BATCH) * UM_BATCH

    # ---------------- mu: dst=users(orig src), src=movies -------------------
    ucore = src // UR
    o_mu = np.argsort(ucore * NU + src, kind="stable")
    s_mu, d_mu, uc2 = src[o_mu], dst[o_mu], ucore[o_mu]
    mu_e = {}
    maxT_mu = 0
    for k in range(NC_):
        m = uc2 == k
        mu_e[k] = (d_mu[m], s_mu[m] - k * UR)
        maxT_mu = max(maxT_mu, _count_tiles(mu_e[k][1], MU_GROUP))
    T_MU = -(-maxT_mu // MU_BATCH) * MU_BATCH

    # ---------------- y: user-sorted slices ---------------------------------
    y_counts = np.bincount(ucore, minlength=NC_)
    T_Y = -(-int(y_counts.max()) // (128 * Y_BATCH)) * Y_BATCH
    orig_ids = np.arange(E)[o_mu]

    per_core, y_orig = [], []
    nb_um, nsb_um = T_UM // UM_BATCH, T_UM // (2 * UM_GROUP)
    nb_mu, nsb_mu = T_MU // MU_BATCH, T_MU // (2 * MU_GROUP)
    nb_y = T_Y // Y_BATCH
    for k in range(NC_):
        d = {}
        gu = np.zeros((NCH, nb_um, 128, UM_BATCH * 8), np.int16)
        gm = np.zeros((NCH, nb_um, 128, UM_BATCH * 8), np.int16)
        sel_l = np.zeros((NCH, nb_um, 128, UM_BATCH, 128), BF)
        sc_l = np.zeros((NCH, nsb_um, 128, 16), np.int16)
        for c in range(NCH):
            gs, gd, sl, sc = _pack_direction(*um_e[(k, c)], T_UM, UM_GROUP, MR)
            for b in range(nb_um):
                t0 = b * UM_BATCH
                gu[c, b] = _wrap16(gs[t0:t0 + UM_BATCH].reshape(-1))
                gm[c, b] = _wrap16((gd[t0:t0 + UM_BATCH] + k * MR).reshape(-1))
            sel_l[c] = sl.reshape(nb_um, UM_BATCH, 128, 128).transpose(0, 2, 1, 3)
            for i in range(nsb_um):
                sc_l[c, i] = _wrap16(sc[2 * i:2 * i + 2].reshape(-1))
        d["um_gu"], d["um_gm"], d["um_sel"], d["um_scat"] = gu, gm, sel_l, sc_l

        gs, gd, sl, sc = _pack_direction(*mu_e[k], T_MU, MU_GROUP, UR)
        gmu_u = np.zeros((nb_mu, 128, MU_BATCH * 8), np.int16)
        gmu_m = np.zeros((nb_mu, 128, MU_BATCH * 8), np.int16)
        for b in range(nb_mu):
            t0 = b * MU_BATCH
            gmu_m[b] = _wrap16(gs[t0:t0 + MU_BATCH].reshape(-1))
            gmu_u[b] = _wrap16(gd[t0:t0 + MU_BATCH].reshape(-1))
        d["mu_gu"], d["mu_gm"] = gmu_u, gmu_m
        d["mu_sel"] = sl.reshape(nb_mu, MU_BATCH, 128, 128).transpose(0, 2, 1, 3).copy()
        d["mu_scat"] = np.stack([_wrap16(sc[2 * i:2 * i + 2].reshape(-1))
                                 for i in range(nsb_mu)])

        m = uc2 == k
        nk = int(m.sum())
        ga = np.zeros(T_Y * 128, np.int64)
        gb = np.zeros(T_Y * 128, np.int64)
        ga[:nk] = s_mu[m] - k * UR
        gb[:nk] = d_mu[m]
        d["y_ga"] = np.stack([_wrap16(ga[i * Y_BATCH * 128:(i + 1) * Y_BATCH * 128])
                              for i in range(nb_y)])
        d["y_gb"] = np.stack([_wrap16(gb[i * Y_BATCH * 128:(i + 1) * Y_BATCH * 128])
                              for i in range(nb_y)])
        y_orig.append(orig_ids[m])
        per_core.append(d)

    meta = dict(T_UM=T_UM, T_MU=T_MU, T_Y=T_Y, y_orig=y_orig, y_counts=y_counts)
    return per_core, meta


# ---------------------------------------------------------------- device build

_CACHE = {}


def _build(T_UM, T_MU, T_Y):
    key = (T_UM, T_MU, T_Y, os.environ.get('KPHASES', 'daMUPY'))
    if key in _CACHE:
        return _CACHE[key]

    nc = bacc.Bacc("TRN2", target_bir_lowering=False)
    dp = nc.declare_dram_parameter

    x_uT_my = dp("x_uT_my", [D, UR], f32, isOutput=False)
    x_mT = dp("x_mT", [D, NM], f32, isOutput=False)
    W_u = dp("W_u", [D, 512], f32, isOutput=False)
    W_m = dp("W_m", [D, 512], f32, isOutput=False)
    B_u = dp("B_u", [P, 512], f32, isOutput=False)
    B_m = dp("B_m", [P, 512], f32, isOutput=False)
    ATT_UM = dp("ATT_UM", [P, 256], bf16, isOutput=False)
    ATT_MU = dp("ATT_MU", [P, 256], bf16, isOutput=False)
    CBIAS = dp("CBIAS", [P, 256], f32, isOutput=False)
    WUF = dp("WUF", [P, 64], f32, isOutput=False)
    WMF = dp("WMF", [P, 64], f32, isOutput=False)
    BUF = dp("BUF", [P, 64], f32, isOutput=False)
    BMF = dp("BMF", [P, 64], f32, isOutput=False)
    WP1U = dp("WP1U", [64, 64], f32, isOutput=False)
    WP1M = dp("WP1M", [64, 64], f32, isOutput=False)
    BP1 = dp("BP1", [P, 64], f32, isOutput=False)
    WP2T = dp("WP2T", [P, 64], f32, isOutput=False)
    IDENT = dp("IDENT", [P, P], f32, isOutput=False)

    nb_um, nsb_um = T_UM // UM_BATCH, T_UM // (2 * UM_GROUP)
    nb_mu, nsb_mu = T_MU // MU_BATCH, T_MU // (2 * MU_GROUP)
    nb_y = T_Y // Y_BATCH
    um_gu = dp("um_gu", [NCH, nb_um, P, UM_BATCH * 8], i16, isOutput=False)
    um_gm = dp("um_gm", [NCH, nb_um, P, UM_BATCH * 8], i16, isOutput=False)
    um_sel = dp("um_sel", [NCH, nb_um, P, UM_BATCH, 128], bf16, isOutput=False)
    um_scat = dp("um_scat", [NCH, nsb_um, P, 16], i16, isOutput=False)
    mu_gu = dp("mu_gu", [nb_mu, P, MU_BATCH * 8], i16, isOutput=False)
    mu_gm = dp("mu_gm", [nb_mu, P, MU_BATCH * 8], i16, isOutput=False)
    mu_sel = dp("mu_sel", [nb_mu, P, MU_BATCH, 128], bf16, isOutput=False)
    mu_scat = dp("mu_scat", [nsb_mu, P, 16], i16, isOutput=False)
    y_ga = dp("y_ga", [nb_y, P, Y_BATCH * 8], i16, isOutput=False)
    y_gb = dp("y_gb", [nb_y, P, Y_BATCH * 8], i16, isOutput=False)

    u_sat_o = dp("u_sat_o", [UR, 64], f32, isOutput=True)
    u_dis_o = dp("u_dis_o", [UR, 64], f32, isOutput=True)
    m_sat_o = dp("m_sat_o", [MR, 64], f32, isOutput=True)
    m_dis_o = dp("m_dis_o", [MR, 64], f32, isOutput=True)
    y_o = dp("y_o", [P, T_Y], f32, isOutput=True)
    stage_u = dp("stage_u", [UR + TRASH, 320], f32, isOutput=True)
    stage_m = dp("stage_m", [NCH, MR + TRASH, 320], f32, isOutput=True)

    ag_in_u = nc.dram_tensor("ag_in_u", [UR, 256], bf16)
    u_proj_um = nc.dram_tensor("u_proj_um", [NU, 256], bf16, addr_space="Shared")
    u_proj_mu = nc.dram_tensor("u_proj_mu", [UR, 256], bf16)
    m_proj_um = nc.dram_tensor("m_proj_um", [NM, 256], bf16)
    m_proj_mu = nc.dram_tensor("m_proj_mu", [NM, 256], bf16)
    a_my = nc.dram_tensor("a_my", [UR, 128], f32)
    b_in = nc.dram_tensor("b_in", [MR, 128], f32)
    b_full = nc.dram_tensor("b_full", [NM, 128], f32, addr_space="Shared")

    with tile.TileContext(nc) as tc:
        with (
            tc.tile_pool(name="const", bufs=1) as cpool,
            tc.tile_pool(name="sbuf", bufs=3) as pool,
            tc.tile_pool(name="psum", bufs=2, space="PSUM") as dpsum,
            tc.tile_pool(name="ppsum", bufs=3, space="PSUM") as ppsum,
            tc.tile_pool(name="spsum", bufs=3, space="PSUM") as spsum,
        ):
            def cload(nm, pr, shape, dt_, conv=None):
                t = cpool.tile(shape, dt_ if conv is None else conv,
                               tag=nm, name=nm)
                if conv is None:
                    nc.sync.dma_start(out=t[:], in_=pr[:])
                else:
                    tmp = pool.tile(shape, dt_, tag="ctmp", name=nm + "_tmp")
                    nc.sync.dma_start(out=tmp[:], in_=pr[:])
                    nc.vector.tensor_copy(out=t[:], in_=tmp[:])
                return t

            W_u_t = cload("cWu", W_u, [D, 512], f32, f32r)
            W_m_t = cload("cWm", W_m, [D, 512], f32, f32r)
            B_u_t = cload("cBu", B_u, [P, 512], f32)
            B_m_t = cload("cBm", B_m, [P, 512], f32)
            att_um_t = cload("cAum", ATT_UM, [P, 256], bf16)
            att_mu_t = cload("cAmu", ATT_MU, [P, 256], bf16)
            cbias_t = cload("cCB", CBIAS, [P, 256], f32)
            wuf_t = cload("cWuf", WUF, [P, 64], f32, f32r)
            wmf_t = cload("cWmf", WMF, [P, 64], f32, f32r)
            buf_t = cload("cbuf", BUF, [P, 64], f32)
            bmf_t = cload("cbmf", BMF, [P, 64], f32)
            wp1u_t = cload("cWp1u", WP1U, [64, 64], f32, f32r)
            wp1m_t = cload("cWp1m", WP1M, [64, 64], f32, f32r)
            bp1_t = cload("cbp1", BP1, [P, 64], f32)
            wp2_t = cload("cWp2", WP2T, [P, 64], f32)
            ident_t = cload("cid", IDENT, [P, P], f32)

            # ---- dense projections ----
            def dense(xT, W_t, B_t, n_rows, writes):
                for i in range(-(-n_rows // P)):
                    r0 = i * P
                    rn = min(P, n_rows - r0)
                    lh = pool.tile([D, P], f32, tag="dlh")
                    nc.sync.dma_start(out=lh[:, :rn], in_=xT[:, r0:r0 + rn])
                    lhr = pool.tile([D, P], f32r, tag="dlhr")
                    nc.vector.tensor_copy(out=lhr[:, :rn], in_=lh[:, :rn])
                    ps = dpsum.tile([P, 512], f32, tag="dps")
                    nc.tensor.matmul(out=ps[:rn, :], lhsT=lhr[:, :rn], rhs=W_t[:],
                                     start=True, stop=True)
                    ob = pool.tile([P, 512], bf16, tag="dob")
                    nc.vector.tensor_tensor(out=ob[:rn, :], in0=ps[:rn, :],
                                            in1=B_t[:rn, :], op=OP.add)
                    for (dst_ap, c0, c1) in writes:
                        nc.sync.dma_start(out=dst_ap[r0:r0 + rn, :],
                                          in_=ob[:rn, c0:c1])

            PHASES = os.environ.get("KPHASES", "daMUPY")
            dense(x_uT_my, W_u_t, B_u_t, UR,
                  [(ag_in_u, 0, 256), (u_proj_mu, 256, 512)])
            if "a" in PHASES:
                nc.gpsimd.collective_compute(
                    "AllGather", OP.bypass, replica_groups=[list(range(NC_))],
                    ins=[ag_in_u[:]], outs=[u_proj_um[:]])
            dense(x_mT, W_m_t, B_m_t, NM,
                  [(m_proj_um, 0, 256), (m_proj_mu, 256, 512)])

            # ---- edge phases ----
            def edge_phase(n_batches, BATCH, GROUP, tab_v, tab_o, vals_is_src,
                           att_tile, gidx_v, gidx_o, sel_p, scat_p, stage_ap):
                NIB = BATCH * 128
                sp_by_g, ssb_by_i = {}, {}
                for b in range(n_batches):
                    gv = pool.tile([P, BATCH, 256], bf16, tag="egv")
                    go = pool.tile([P, BATCH, 256], bf16, tag="ego")
                    iv = pool.tile([P, BATCH * 8], i16, tag="eiv")
                    io = pool.tile([P, BATCH * 8], i16, tag="eio")
                    nc.sync.dma_start(out=iv[:], in_=gidx_v[b])
                    nc.sync.dma_start(out=io[:], in_=gidx_o[b])
                    nc.gpsimd.dma_gather(out_ap=gv[:], in_ap=tab_v, idxs_ap=iv[:],
                                         num_idxs=NIB, num_idxs_reg=NIB,
                                         elem_size=256)
                    nc.gpsimd.dma_gather(out_ap=go[:], in_ap=tab_o, idxs_ap=io[:],
                                         num_idxs=NIB, num_idxs_reg=NIB,
                                         elem_size=256)
                    selt = pool.tile([P, BATCH * 128], bf16, tag="esel")
                    nc.sync.dma_start(out=selt[:],
                                      in_=sel_p[b].rearrange("p a b -> p (a b)"))

                    z = pool.tile([P, BATCH, 256], bf16, tag="ez")
                    nc.vector.tensor_tensor(out=z[:], in0=gv[:], in1=go[:], op=OP.add)
                    m08 = pool.tile([P, BATCH, 256], bf16, tag="em08")
                    nc.vector.tensor_scalar(out=m08[:], in0=z[:], scalar1=0.0,
                                            scalar2=0.8, op0=OP.min, op1=OP.mult)
                    nc.vector.tensor_tensor(out=z[:], in0=z[:], in1=m08[:],
                                            op=OP.subtract)
                    ta = m08
                    nc.vector.tensor_tensor(
                        out=ta[:], in0=z[:],
                        in1=att_tile[:, None, :].to_broadcast([P, BATCH, 256]),
                        op=OP.mult)
                    lg = pool.tile([P, BATCH * 4], f32, tag="elg")
                    nc.vector.tensor_reduce(
                        out=lg[:], in_=ta[:].rearrange("p a (h c) -> p (a h) c", h=4),
                        axis=mybir.AxisListType.X, op=OP.add)
                    exf = pool.tile([P, BATCH * 4], f32, tag="eexf")
                    nc.scalar.activation(out=exf[:], in_=lg[:], func=AF.Exp)
                    exw = pool.tile([P, BATCH, 260], bf16, tag="eexw")
                    nc.vector.tensor_copy(
                        out=exw[:, :, 0:4],
                        in_=exf[:].rearrange("p (a h) -> p a h", h=4))
                    vals = gv if vals_is_src else go
                    nc.vector.tensor_tensor(
                        out=exw[:, :, 4:260].rearrange("p a (h c) -> p a h c", h=4),
                        in0=vals[:].rearrange("p a (h c) -> p a h c", h=4),
                        in1=exf[:].rearrange("p (a h) -> p a h", h=4)[:, :, :, None]
                            .to_broadcast([P, BATCH, 4, 64]),
                        op=OP.mult)
                    for t in range(BATCH):
                        gt = b * BATCH + t
                        gi = gt // GROUP
                        first = (gt % GROUP) == 0
                        last = (gt % GROUP) == GROUP - 1
                        if first:
                            sp_by_g[gi] = spsum.tile([P, 260], f32, tag="sp", name=f"sp{gi}")
                        nc.tensor.matmul(out=sp_by_g[gi][:],
                                         lhsT=selt[:, t * 128:(t + 1) * 128],
                                         rhs=exw[:, t, :], start=first, stop=last)
                        if last:
                            sb_i, half = gi // 2, gi % 2
                            if half == 0:
                                ssb_by_i[sb_i] = pool.tile([P, 2, 260], f32, tag="essb", name=f"ssb{sb_i}")
                            nc.scalar.copy(out=ssb_by_i[sb_i][:, half, :],
                                           in_=sp_by_g[gi][:])
                            if half == 1:
                                sidx = pool.tile([P, 16], i16, tag="esidx")
                                nc.sync.dma_start(out=sidx[:], in_=scat_p[sb_i])
                                nc.gpsimd.dma_scatter_add(
                                    out_ap=stage_ap[:, 0:260],
                                    in_ap=ssb_by_i[sb_i][:],
                                    idxs_ap=sidx[:], num_idxs=256,
                                    num_idxs_reg=256, elem_size=260,
                                    elem_step=320)

            if "M" in PHASES:
                edge_phase(nb_mu, MU_BATCH, MU_GROUP, m_proj_mu[:, :],
                           u_proj_mu[:, :], True, att_mu_t, mu_gm, mu_gu,
                           mu_sel, mu_scat, stage_u[:, :])
            for c in (range(1 if "1" in PHASES else NCH) if "U" in PHASES else []):
                um_src = (m_proj_um[:, :] if "G" in PHASES
                          else u_proj_um[c * UCH:(c + 1) * UCH, :])
                edge_phase(nb_um, UM_BATCH, UM_GROUP,
                           um_src, m_proj_um[:, :],
                           True, att_um_t, um_gu[c], um_gm[c], um_sel[c],
                           um_scat[c],
                           stage_u[:, :] if "S" in PHASES else stage_m[c, :, :])

            # ---- post phase ----
            def post(stage_list, n_rows, cb0, att_out, dis_out, Wf_t, Bf_t,
                     wp1_t, a_dst, bias_a):
                for i in range(-(-n_rows // P)):
                    r0 = i * P
                    rn = min(P, n_rows - r0)
                    st = pool.tile([P, 260], f32, tag="pst")
                    nc.sync.dma_start(out=st[:rn, :],
                                      in_=stage_list[0][r0:r0 + rn, 0:260])
                    for sx in stage_list[1:]:
                        st2 = pool.tile([P, 260], f32, tag="pst2")
                        nc.sync.dma_start(out=st2[:rn, :],
                                          in_=sx[r0:r0 + rn, 0:260])
                        nc.vector.tensor_tensor(out=st[:rn, :], in0=st[:rn, :],
                                                in1=st2[:rn, :], op=OP.add)
                    inv = pool.tile([P, 4], f32, tag="pinv")
                    nc.vector.tensor_scalar_add(out=inv[:rn, :], in0=st[:rn, 0:4],
                                                scalar1=1e-16)
                    nc.vector.reciprocal(out=inv[:rn, :], in_=inv[:rn, :])
                    nc.vector.tensor_scalar_mul(out=inv[:rn, :], in0=inv[:rn, :],
                                                scalar1=0.5)
                    A = pool.tile([P, 4, 64], f32, tag="pA")
                    nc.vector.tensor_tensor(
                        out=A[:rn],
                        in0=st[:rn, 4:260].rearrange("p (h c) -> p h c", h=4),
                        in1=inv[:rn, :, None].to_broadcast([rn, 4, 64]),
                        op=OP.mult)
                    ucat = pool.tile([P, 128], f32, tag="pucat")
                    if rn < P:
                        nc.gpsimd.memset(ucat[:], 0.0)
                    nc.vector.tensor_tensor(out=ucat[:rn, 0:64], in0=A[:rn, 0, :],
                                            in1=A[:rn, 1, :], op=OP.add)
                    nc.vector.tensor_tensor(out=ucat[:rn, 0:64],
                                            in0=ucat[:rn, 0:64],
                                            in1=cbias_t[:rn, cb0:cb0 + 64],
                                            op=OP.add)
                    nc.vector.tensor_tensor(out=ucat[:rn, 64:128], in0=A[:rn, 2, :],
                                            in1=A[:rn, 3, :], op=OP.add)
                    nc.vector.tensor_tensor(out=ucat[:rn, 64:128],
                                            in0=ucat[:rn, 64:128],
                                            in1=cbias_t[:rn, cb0 + 64:cb0 + 128],
                                            op=OP.add)
                    nc.sync.dma_start(out=att_out[r0:r0 + rn, :],
                                      in_=ucat[:rn, 0:64])
                    nc.sync.dma_start(out=dis_out[r0:r0 + rn, :],
                                      in_=ucat[:rn, 64:128])
                    tp = ppsum.tile([P, P], f32, tag="pp")
                    nc.tensor.transpose(out=tp[:], in_=ucat[:], identity=ident_t[:])
                    tpr = pool.tile([P, P], f32r, tag="ptpr")
                    nc.scalar.copy(out=tpr[:], in_=tp[:])
                    fp = ppsum.tile([P, 64], f32, tag="pp")
                    nc.tensor.matmul(out=fp[:, :], lhsT=tpr[:, :], rhs=Wf_t[:],
                                     start=True, stop=True)
                    uf = pool.tile([P, 64], f32, tag="puf")
                    nc.vector.tensor_tensor(out=uf[:], in0=fp[:, :],
                                            in1=Bf_t[:], op=OP.add)
                    nc.vector.tensor_scalar_max(out=uf[:], in0=uf[:], scalar1=0.0)
                    tp2 = ppsum.tile([P, P], f32, tag="pp")
                    nc.tensor.transpose(out=tp2[:64, :], in_=uf[:, 0:64],
                                        identity=ident_t[:])
                    tp2r = pool.tile([64, P], f32r, tag="ptp2r")
                    nc.scalar.copy(out=tp2r[:], in_=tp2[:64, :])
                    ap_ = ppsum.tile([P, 64], f32, tag="pp")
                    nc.tensor.matmul(out=ap_[:, :], lhsT=tp2r[:, :], rhs=wp1_t[:],
                                     start=True, stop=True)
                    av = pool.tile([P, 64], f32, tag="pav")
                    if bias_a is not None:
                        nc.vector.tensor_tensor(out=av[:rn], in0=ap_[:rn, :],
                                                in1=bias_a[:rn], op=OP.add)
                    else:
                        nc.vector.tensor_copy(out=av[:rn], in_=ap_[:rn, :])
                    nc.sync.dma_start(out=a_dst[r0:r0 + rn, 0:64], in_=av[:rn])

            if "P" in PHASES:
                post([stage_u], UR, 0, u_sat_o, u_dis_o, wuf_t, buf_t, wp1u_t,
                     a_my, None)
                post([stage_m[c] for c in range(NCH)], MR, 128, m_sat_o,
                     m_dis_o, wmf_t, bmf_t, wp1m_t, b_in, bp1_t)
            if "a" in PHASES:
                nc.gpsimd.collective_compute(
                    "AllGather", OP.bypass, replica_groups=[list(range(NC_))],
                    ins=[b_in[:]], outs=[b_full[:]])

            # ---- y phase ----
            for b in (range(nb_y) if "Y" in PHASES else []):
                ia = pool.tile([P, Y_BATCH * 8], i16, tag="yia")
                ib = pool.tile([P, Y_BATCH * 8], i16, tag="yib")
                nc.sync.dma_start(out=ia[:], in_=y_ga[b])
                nc.sync.dma_start(out=ib[:], in_=y_gb[b])
                ga = pool.tile([P, Y_BATCH, 128], f32, tag="yga")
                gb = pool.tile([P, Y_BATCH, 128], f32, tag="ygb")
                nc.gpsimd.dma_gather(out_ap=ga[:], in_ap=a_my[:, :], idxs_ap=ia[:],
                                     num_idxs=Y_BATCH * 128,
                                     num_idxs_reg=Y_BATCH * 128, elem_size=128)
                nc.gpsimd.dma_gather(
                    out_ap=gb[:], in_ap=b_full[:, :],
                    idxs_ap=ib[:], num_idxs=Y_BATCH * 128,
                    num_idxs_reg=Y_BATCH * 128, elem_size=128)
                z = pool.tile([P, Y_BATCH, 64], f32, tag="yz")
                nc.vector.tensor_tensor(out=z[:], in0=ga[:, :, 0:64],
                                        in1=gb[:, :, 0:64], op=OP.add)
                nc.vector.tensor_scalar_max(out=z[:], in0=z[:], scalar1=0.0)
                nc.vector.tensor_tensor(
                    out=z[:], in0=z[:],
                    in1=wp2_t[:, None, :].to_broadcast([P, Y_BATCH, 64]),
                    op=OP.mult)
                yv = pool.tile([P, Y_BATCH], f32, tag="yv")
                nc.vector.tensor_reduce(out=yv[:], in_=z[:],
                                        axis=mybir.AxisListType.X, op=OP.add)
                nc.sync.dma_start(out=y_o[:, b * Y_BATCH:(b + 1) * Y_BATCH],
                                  in_=yv[:])

    nc.compile()
    _CACHE[key] = nc
    return nc


# ---------------------------------------------------------------- entry point

def kernel(**inputs):
    src = np.asarray(inputs["src"]).astype(np.int64)
    dst = np.asarray(inputs["dst"]).astype(np.int64)
    f = lambda n: np.asarray(inputs[n]).astype(np.float32)

    per_core, meta = _prep(src, dst)
    nc = _build(meta["T_UM"], meta["T_MU"], meta["T_Y"])

    def cat_w(*names):
        return np.concatenate([f(n) for n in names], axis=1)

    common = {
        "x_mT": np.ascontiguousarray(f("x_m").T),
        "W_u": cat_w("Wl_um_sat", "Wl_um_dis", "Wr_mu_sat", "Wr_mu_dis"),
        "W_m": cat_w("Wr_um_sat", "Wr_um_dis", "Wl_mu_sat", "Wl_mu_dis"),
        "B_u": np.tile(np.concatenate([f("bl_um_sat"), f("bl_um_dis"),
                                       f("br_mu_sat"), f("br_mu_dis")]), (P, 1)),
        "B_m": np.tile(np.concatenate([f("br_um_sat"), f("br_um_dis"),
                                       f("bl_mu_sat"), f("bl_mu_dis")]), (P, 1)),
        "ATT_UM": np.tile(np.concatenate([f("att_um_sat").reshape(-1),
                                          f("att_um_dis").reshape(-1)]),
                          (P, 1)).astype(BF),
        "ATT_MU": np.tile(np.concatenate([f("att_mu_sat").reshape(-1),
                                          f("att_mu_dis").reshape(-1)]),
                          (P, 1)).astype(BF),
        "CBIAS": np.tile(np.concatenate([f("bias_mu_sat"), f("bias_mu_dis"),
                                         f("bias_um_sat"), f("bias_um_dis")]),
                         (P, 1)),
        "WUF": f("Wuf"), "WMF": f("Wmf"),
        "BUF": np.tile(f("buf"), (P, 1)), "BMF": np.tile(f("bmf"), (P, 1)),
        "WP1U": np.ascontiguousarray(f("Wp1")[:64]),
        "WP1M": np.ascontiguousarray(f("Wp1")[64:]),
        "BP1": np.tile(f("bp1"), (P, 1)),
        "WP2T": np.tile(f("Wp2")[:, 0], (P, 1)),
        "IDENT": np.eye(P, dtype=np.float32),
    }
    x_uT = np.ascontiguousarray(f("x_u").T)
    in_maps = []
    for k in range(NC_):
        m = dict(common)
        m["x_uT_my"] = np.ascontiguousarray(x_uT[:, k * UR:(k + 1) * UR])
        m.update(per_core[k])
        in_maps.append(m)

    trace = os.environ.get("KTRACE", "0") == "1"
    res = run_bass_kernel_spmd(nc, in_maps, core_ids=list(range(NC_)),
                               trace=trace)
    kernel.last_exec_time_ns = getattr(res, "exec_time_ns", None)
    kernel.last_profile = res
    r = res.results

    u_sat = np.concatenate([r[k]["u_sat_o"] for k in range(NC_)], axis=0)
    u_dis = np.concatenate([r[k]["u_dis_o"] for k in range(NC_)], axis=0)
    m_sat = np.concatenate([r[k]["m_sat_o"] for k in range(NC_)], axis=0)
    m_dis = np.concatenate([r[k]["m_dis_o"] for k in range(NC_)], axis=0)
    y = np.empty(E, np.float32)
    bp2 = float(f("bp2")[0])
    for k in range(NC_):
        nk = int(meta["y_counts"][k])
        y[meta["y_orig"][k]] = r[k]["y_o"].T.reshape(-1)[:nk] + bp2
    return y, (u_sat, u_dis, m_sat, m_dis)
